# revision 1
# baseline (speedup 1.0000x reference)
"""Trainium2 Bass kernel for nn_DeformLikeASPPConv (8-core data parallel).

Self-contained: kernel(**inputs) takes the full-batch inputs and returns the
full output. One sample per NeuronCore. See emit() for the device pipeline.
"""
import sys
if "/opt/trn_rl_repo" not in sys.path:
    sys.path.insert(0, "/opt/trn_rl_repo")
import numpy as np
import ml_dtypes
import concourse.bass as bass
import concourse.bacc as bacc
import concourse.tile as tile
import concourse.mybir as mybir
from concourse import bass_utils

N_CORES = 8
H, W = 256, 256


NPBF16 = ml_dtypes.bfloat16
C = 64
DIL = 12
BN_EPS = 1e-5


def conv2d_np(x, w, dilation, padding):
    # x: [Cin, H, W], w: [Cout, Cin, 3, 3]
    cin, H, W = x.shape
    cout = w.shape[0]
    p = padding
    xp = np.zeros((cin, H + 2 * p, W + 2 * p), np.float32)
    xp[:, p:p + H, p:p + W] = x
    y = np.zeros((cout, H, W), np.float32)
    for r in range(3):
        for s in range(3):
            sh = xp[:, r * dilation:r * dilation + H,
                    s * dilation:s * dilation + W]
            y += np.einsum('oc,chw->ohw', w[:, :, r, s], sh,
                           optimize=True).astype(np.float32)
    return y


def reference_np(x, offset_w, offset_b, conv_w, bn_gamma, bn_beta, bn_mean,
                 bn_var):
    # x: [C, H, W] single sample
    c, H, W = x.shape
    off = conv2d_np(x, offset_w, 1, 1) + offset_b[:, None, None]
    off = np.tanh(off) * 2.0
    gx = np.linspace(-1.0, 1.0, W, dtype=np.float32)
    gy = np.linspace(-1.0, 1.0, H, dtype=np.float32)
    sx = gx[None, :] + off[0] / max(float(W - 1), 1.0) * 2.0
    sy = gy[:, None] + off[1] / max(float(H - 1), 1.0) * 2.0
    ix = np.clip((sx + 1) * (W - 1) * 0.5, 0, W - 1)
    iy = np.clip((sy + 1) * (H - 1) * 0.5, 0, H - 1)
    x0 = np.floor(ix).astype(np.int32)
    y0 = np.floor(iy).astype(np.int32)
    wx = ix - x0
    wy = iy - y0
    x1 = np.minimum(x0 + 1, W - 1)
    y1 = np.minimum(y0 + 1, H - 1)
    v00 = x[:, y0, x0]
    v01 = x[:, y0, x1]
    v10 = x[:, y1, x0]
    v11 = x[:, y1, x1]
    top = v00 * (1 - wx) + v01 * wx
    bot = v10 * (1 - wx) + v11 * wx
    warped = top * (1 - wy) + bot * wy
    y = conv2d_np(warped.astype(np.float32), conv_w, DIL, DIL)
    inv = bn_gamma / np.sqrt(bn_var + BN_EPS)
    y = y * inv[:, None, None] + (bn_beta - bn_mean * inv)[:, None, None]
    return np.maximum(y, 0)


def prep_core_inputs(x, offset_w, offset_b, conv_w, bn_gamma, bn_beta,
                     bn_mean, bn_var):
    """x: [C, H, W] fp32 one sample -> dict of kernel inputs."""
    c, H, W = x.shape
    N = H * W
    CF = N // 128
    x_cm = x.reshape(C, N).astype(np.float32)
    pm = np.ascontiguousarray(x.reshape(C, N).T).astype(NPBF16)  # [N, C]
    p = np.arange(N)
    x_pm4 = np.concatenate([
        pm[np.minimum(p + d, N - 1)] for d in (0, 1, W, W + 1)],
        axis=1)  # [N, 4C]
    wo18 = np.zeros((C, 18), np.float32)
    for t in range(9):
        r, s = t // 3, t % 3
        for o in range(2):
            wo18[:, 2 * t + o] = offset_w[o, :, r, s]
    sel18 = np.zeros((18, 2), np.float32)
    for t in range(9):
        for o in range(2):
            sel18[2 * t + o, o] = 1.0
    inv = (bn_gamma / np.sqrt(bn_var + BN_EPS)).astype(np.float32)
    wmf = conv_w * inv[:, None, None, None]  # [Cout, Cin, 3, 3]
    wm1 = np.zeros((C, 3 * C), np.float32)
    wm1a = np.zeros((C, 3 * C), np.float32)
    wm1b = np.zeros((C, 3 * C), np.float32)
    wm2 = np.zeros((2 * C, 3 * C), np.float32)
    for gs in range(3):  # ds index 0,1,2 -> shift -1,0,+1
        wm1[:, gs * C:(gs + 1) * C] = wmf[:, :, 1, gs].T
        wm1a[:, gs * C:(gs + 1) * C] = wmf[:, :, 0, gs].T
        wm1b[:, gs * C:(gs + 1) * C] = wmf[:, :, 2, gs].T
        wm2[0:C, gs * C:(gs + 1) * C] = wmf[:, :, 0, gs].T
        wm2[C:2 * C, gs * C:(gs + 1) * C] = wmf[:, :, 2, gs].T
    # order in kern.py: gcol = (ds+1)*C with ds in (0,-1,1) -> gs = ds+1
    biasy = (bn_beta - bn_mean * inv).astype(np.float32).reshape(C, 1)
    pix = np.arange(N).reshape(128, CF)  # compact: partition p -> pixels
    jmap = (pix % W).astype(np.float32)
    imap = (pix // W).astype(np.float32)
    return {
        "x_cm": x_cm,
        "x_pm4": x_pm4,
        "wo18": wo18.astype(np.float32),
        "sel18": sel18.astype(np.float32),
        "wm1": wm1.astype(NPBF16),
        "wm1a": wm1a.astype(NPBF16),
        "wm1b": wm1b.astype(NPBF16),
        "wm2": wm2.astype(NPBF16),
        "offb": offset_b.astype(np.float32).reshape(2, 1),
        "biasy": biasy,
        "jmap": jmap,
        "imap": imap,
    }


IN_SPECS = [
    ("x_cm", (C, None), np.float32),
    ("x_pm4", (None, 4 * C), NPBF16),
    ("wo18", (C, 18), np.float32),
    ("sel18", (18, 2), np.float32),
    ("wm1", (C, 3 * C), NPBF16),
    ("wm1a", (C, 3 * C), NPBF16),
    ("wm1b", (C, 3 * C), NPBF16),
    ("wm2", (2 * C, 3 * C), NPBF16),
    ("offb", (2, 1), np.float32),
    ("biasy", (C, 1), np.float32),
    ("jmap", (128, None), np.float32),
    ("imap", (128, None), np.float32),
]



F32 = mybir.dt.float32
BF16 = mybir.dt.bfloat16
I16 = mybir.dt.int16
I32 = mybir.dt.int32
ALU = mybir.AluOpType
AF = mybir.ActivationFunctionType

C = 64
DIL = 12


def emit(tc, io, H, W):
    nc = tc.nc
    N = H * W
    CF = N // 128
    Po = W + 2
    R_A = 16
    R_B = 8
    M_B = R_B * W
    CLX = (W - 2) + 0.99609375
    CLY = (H - 2) + 0.99609375

    x_cm, x_pm4 = io["x_cm"], io["x_pm4"]
    wo18, sel18 = io["wo18"], io["sel18"]
    wm1, wm1a, wm1b, wm2 = io["wm1"], io["wm1a"], io["wm1b"], io["wm2"]
    offb, biasy = io["offb"], io["biasy"]
    jmap, imap = io["jmap"], io["imap"]
    y_out = io["y"]

    with tc.tile_pool(name="dram", bufs=1, space="DRAM") as dramp, \
         tc.tile_pool(name="consts", bufs=1) as cstp:
        ox_dram = dramp.tile([2, N], F32)
        wxl = dramp.tile([1, N], BF16)
        wyl = dramp.tile([1, N], BF16)
        idxl = dramp.tile([1, N], F32)

        offb_s = cstp.tile([2, 1], F32, tag="offb")
        nc.sync.dma_start(offb_s[:], offb[:])
        biasy_s = cstp.tile([C, 1], F32, tag="biasy")
        nc.sync.dma_start(biasy_s[:], biasy[:])
        wo18_s = cstp.tile([C, 18], F32, tag="wo18")
        nc.sync.dma_start(wo18_s[:], wo18[:])
        sel18_s = cstp.tile([18, 2], F32, tag="sel18")
        nc.sync.dma_start(sel18_s[:], sel18[:])

        # ---------------- Phase A: offset head ----------------
        with tc.tile_pool(name="xa", bufs=2) as xap, \
             tc.tile_pool(name="o18", bufs=2) as o18p, \
             tc.tile_pool(name="al", bufs=2) as alp, \
             tc.tile_pool(name="oxs", bufs=2) as oxsp, \
             tc.tile_pool(name="psA", bufs=3, space="PSUM") as psA, \
             tc.tile_pool(name="psA2", bufs=3, space="PSUM") as psA2:
            for r0 in range(0, H, R_A):
                lo = max(0, r0 - 1)
                hi = min(H, r0 + R_A + 1)
                xt = xap.tile([C, (R_A + 2) * W], F32, tag="xa")
                nc.sync.dma_start(xt[:, 0:(hi - lo) * W],
                                  x_cm[:, lo * W:hi * W])
                o18s = o18p.tile([18, (R_A + 2) * Po], F32, tag="o18")
                o18v = o18s[:].rearrange("p (r w) -> p r w", w=Po)
                nc.vector.memset(o18v[:, :, 0:1], 0.0)
                nc.vector.memset(o18v[:, :, Po - 1:Po], 0.0)
                if r0 == 0:
                    nc.vector.memset(o18v[:, 0:1, :], 0.0)
                if r0 + R_A >= H:
                    nc.vector.memset(o18v[:, R_A + 1:R_A + 2, :], 0.0)
                for cr in range(lo, hi, 2):
                    nrr = min(2, hi - cr)
                    ps = psA.tile([18, 2 * W], F32, tag="psA")
                    nc.tensor.matmul(ps[:, 0:nrr * W], wo18_s[:],
                                     xt[:, (cr - lo) * W:(cr - lo + nrr) * W],
                                     start=True, stop=True)
                    srow = cr - r0 + 1
                    nc.scalar.activation(
                        o18v[:, srow:srow + nrr, 1:W + 1], ps[:, 0:nrr * W],
                        AF.Copy, scale=1.0)
                al = alp.tile([18, R_A * W], F32, tag="al")
                for t in range(9):
                    dr, ds = t // 3 - 1, t % 3 - 1
                    nc.sync.dma_start(
                        al[2 * t:2 * t + 2, :].rearrange(
                            "p (r w) -> p r w", w=W),
                        o18v[2 * t:2 * t + 2, 1 + dr:1 + dr + R_A,
                             1 + ds:1 + ds + W])
                oxs = oxsp.tile([2, R_A * W], F32, tag="oxs")
                for c0 in range(0, R_A * W, 512):
                    ps2 = psA2.tile([2, 512], F32, tag="psA2")
                    nc.tensor.matmul(ps2[:], sel18_s[:], al[:, c0:c0 + 512],
                                     start=True, stop=True)
                    nc.scalar.activation(oxs[:, c0:c0 + 512], ps2[:],
                                         AF.Tanh, bias=offb_s[:], scale=1.0)
                nc.sync.dma_start(ox_dram[:, r0 * W:(r0 + R_A) * W], oxs[:])

        # ---------------- Maps (compact [128, CF]) ----------------
        with tc.tile_pool(name="mp", bufs=1) as mp:
            jm = mp.tile([128, CF], F32, tag="jm")
            nc.sync.dma_start(jm[:], jmap[:])
            im = mp.tile([128, CF], F32, tag="im")
            nc.sync.dma_start(im[:], imap[:])

            def coord_chain(row, base_map, clmax, wl_dram):
                oc = mp.tile([128, CF], F32, tag=f"oc{row}")
                nc.sync.dma_start(
                    oc[:], bass.AP(tensor=ox_dram[:].tensor,
                                   offset=ox_dram[:].offset + row * N,
                                   ap=[[CF, 128], [1, CF]]))
                ic = mp.tile([128, CF], F32, tag=f"ic{row}")
                nc.vector.scalar_tensor_tensor(ic[:], oc[:], 2.0, base_map[:],
                                               ALU.mult, ALU.add)
                nc.vector.tensor_scalar(ic[:], ic[:], 0.0, clmax,
                                        ALU.max, ALU.min)
                i32 = mp.tile([128, CF], I32, tag=f"i32{row}")
                nc.vector.tensor_copy(i32[:], ic[:])
                c0f = mp.tile([128, CF], F32, tag=f"c0f{row}")
                nc.vector.tensor_copy(c0f[:], i32[:])
                wf = mp.tile([128, CF], F32, tag=f"wf{row}")
                nc.vector.tensor_tensor(wf[:], ic[:], c0f[:], ALU.subtract)
                msk = mp.tile([128, CF], F32, tag=f"msk{row}")
                nc.vector.tensor_scalar(msk[:], wf[:], 0.0, None, ALU.is_lt)
                nc.vector.tensor_tensor(c0f[:], c0f[:], msk[:], ALU.subtract)
                nc.vector.tensor_tensor(wf[:], ic[:], c0f[:], ALU.subtract)
                wb = mp.tile([128, CF], BF16, tag=f"wb{row}")
                nc.vector.tensor_copy(wb[:], wf[:])
                nc.sync.dma_start(wl_dram[:], wb[:])
                return c0f

            x0f = coord_chain(0, jm, CLX, wxl)
            y0f = coord_chain(1, im, CLY, wyl)
            idxf = mp.tile([128, CF], F32, tag="idxf")
            nc.vector.scalar_tensor_tensor(idxf[:], y0f[:], float(W), x0f[:],
                                           ALU.mult, ALU.add)
            nc.sync.dma_start(idxl[:], idxf[:])

        # ---------------- Phase B: gather + combine ----------------
        with tc.tile_pool(name="w2", bufs=1) as w2p:
            W2 = w2p.tile([128, N + 2 * W], BF16, tag="W2")
            with tc.tile_pool(name="gb", bufs=2) as gbp, \
                 tc.tile_pool(name="wtb", bufs=2) as wtp, \
                 tc.tile_pool(name="ixb", bufs=2) as ixp, \
                 tc.tile_pool(name="lcb", bufs=2) as lcp:
                for r0 in range(0, H, R_B):
                    base_px = max(0, r0 - 2) * W
                    idxt = ixp.tile([16, M_B // 16], F32, tag="ixf")
                    nc.sync.dma_start(
                        idxt[:], bass.AP(tensor=idxl[:].tensor,
                                         offset=idxl[:].offset + r0 * W,
                                         ap=[[1, 16], [16, M_B // 16]]))
                    sep = ixp.tile([16, M_B // 16], F32, tag="ixs")
                    nc.vector.tensor_scalar(sep[:], idxt[:], float(base_px),
                                            None, ALU.subtract)
                    i16 = ixp.tile([128, M_B // 16], I16, tag="ix16")
                    nc.vector.tensor_copy(i16[0:16, :], sep[:])
                    for rep in range(1, 8):
                        nc.sync.dma_start(i16[16 * rep:16 * rep + 16, :],
                                          i16[0:16, :])
                    g = gbp.tile([128, 2, M_B], BF16, tag="g")
                    nc.gpsimd.dma_gather(
                        g[:], bass.AP(tensor=x_pm4[:].tensor,
                                      offset=x_pm4[:].offset + base_px * 4 * C,
                                      ap=[[4 * C, N - base_px], [1, 4 * C]]),
                        i16[:], M_B, M_B, 4 * C, transpose=True,
                        single_packet=False)
                    wyt = wtp.tile([128, M_B], BF16, tag="wy")
                    nc.sync.dma_start(
                        wyt[:], bass.AP(tensor=wyl[:].tensor,
                                        offset=wyl[:].offset + r0 * W,
                                        ap=[[0, 128], [1, M_B]]))
                    wxt = wtp.tile([64, M_B], BF16, tag="wx")
                    nc.sync.dma_start(
                        wxt[:], bass.AP(tensor=wxl[:].tensor,
                                        offset=wxl[:].offset + r0 * W,
                                        ap=[[0, 64], [1, M_B]]))
                    g0 = g[:, 0, :]
                    g1 = g[:, 1, :]
                    nc.vector.tensor_tensor(g1, g1, g0, ALU.subtract)
                    nc.vector.tensor_tensor(g1, g1, wyt[:], ALU.mult)
                    nc.vector.tensor_tensor(g0, g0, g1, ALU.add)
                    l0 = g[0:64, 0, :]
                    lc = lcp.tile([64, M_B], BF16, tag="lc")
                    nc.scalar.copy(lc[:], g[64:128, 0, :])
                    nc.vector.tensor_tensor(lc[:], lc[:], l0, ALU.subtract)
                    nc.vector.tensor_tensor(lc[:], lc[:], wxt[:], ALU.mult)
                    nc.vector.tensor_tensor(
                        W2[0:64, r0 * W:(r0 + R_B) * W], l0, lc[:], ALU.add)
                    blo = max(2 * DIL, r0)
                    if blo < r0 + R_B:
                        nc.vector.tensor_copy(
                            W2[64:128,
                               (blo - 2 * DIL) * W:(r0 + R_B - 2 * DIL) * W],
                            W2[0:64, blo * W:(r0 + R_B) * W])

            # ---------------- Dilated conv + BN + ReLU ----------------
            with tc.tile_pool(name="wc", bufs=1) as wc, \
                 tc.tile_pool(name="yb", bufs=3) as ybp, \
                 tc.tile_pool(name="psC", bufs=4, space="PSUM") as psC:
                wm1_s = wc.tile([C, 3 * C], BF16, tag="wm1")
                nc.sync.dma_start(wm1_s[:], wm1[:])
                wm1a_s = wc.tile([C, 3 * C], BF16, tag="wm1a")
                nc.sync.dma_start(wm1a_s[:], wm1a[:])
                wm1b_s = wc.tile([C, 3 * C], BF16, tag="wm1b")
                nc.sync.dma_start(wm1b_s[:], wm1b[:])
                wm2_s = wc.tile([128, 3 * C], BF16, tag="wm2")
                nc.sync.dma_start(wm2_s[:], wm2[:])
                yb = None
                for r in range(H):
                    if r % 8 == 0:
                        yb = ybp.tile([C, 8 * W], F32, tag="yb")
                    ps = psC.tile([C, W], F32, tag="psC")
                    seg = {-1: (DIL, W, -DIL), 0: (0, W, 0), 1: (0, W - DIL, DIL)}
                    mms = []
                    for ds in (0, -1, 1):
                        olo, ohi, dsoff = seg[ds]
                        gcol = (ds + 1) * C
                        mms.append((ps[:, olo:ohi], wm1_s[:, gcol:gcol + C],
                                    W2[0:64, r * W + olo + dsoff:
                                       r * W + ohi + dsoff]))
                        if DIL <= r < H - DIL:
                            mms.append(
                                (ps[:, olo:ohi], wm2_s[:, gcol:gcol + C],
                                 W2[:, (r - DIL) * W + olo + dsoff:
                                    (r - DIL) * W + ohi + dsoff]))
                        elif r < DIL:
                            mms.append(
                                (ps[:, olo:ohi], wm1b_s[:, gcol:gcol + C],
                                 W2[0:64, (r + DIL) * W + olo + dsoff:
                                    (r + DIL) * W + ohi + dsoff]))
                        else:
                            mms.append(
                                (ps[:, olo:ohi], wm1a_s[:, gcol:gcol + C],
                                 W2[0:64, (r - DIL) * W + olo + dsoff:
                                    (r - DIL) * W + ohi + dsoff]))
                    for k, (o, l, rr) in enumerate(mms):
                        nc.tensor.matmul(o, l, rr, start=(k == 0),
                                         stop=(k == len(mms) - 1))
                    nc.scalar.activation(yb[:, (r % 8) * W:(r % 8 + 1) * W],
                                         ps[:], AF.Relu, bias=biasy_s[:],
                                         scale=1.0)
                    if r % 8 == 7:
                        nc.sync.dma_start(y_out[:, (r - 7) * W:(r + 1) * W],
                                          yb[:])


_NC_CACHE = {}


def build_nc():
    if "nc" in _NC_CACHE:
        return _NC_CACHE["nc"]
    nc = bacc.Bacc("TRN2", target_bir_lowering=False, debug=False,
                   num_devices=N_CORES)
    N = H * W
    CF = N // 128
    io = {}
    for name, shape, dt in IN_SPECS:
        shape = tuple(s if s is not None else
                      (N if name != "jmap" and name != "imap" else CF)
                      for s in shape)
        mdt = {np.float32: mybir.dt.float32}.get(dt, None)
        if dt is NPBF16:
            mdt = mybir.dt.bfloat16
        elif dt is np.float32:
            mdt = mybir.dt.float32
        io[name] = nc.dram_tensor(name, list(shape), mdt,
                                  kind="ExternalInput").ap()
    io["y"] = nc.dram_tensor("y", [C, N], mybir.dt.float32,
                             kind="ExternalOutput").ap()
    with tile.TileContext(nc) as tc:
        emit(tc, io, H, W)
    nc.compile()
    _NC_CACHE["nc"] = nc
    return nc


def kernel(x, offset_w, offset_b, conv_w, bn_gamma, bn_beta, bn_mean, bn_var):
    x = np.asarray(x, np.float32)
    offset_w = np.asarray(offset_w, np.float32)
    offset_b = np.asarray(offset_b, np.float32)
    conv_w = np.asarray(conv_w, np.float32)
    bn_gamma = np.asarray(bn_gamma, np.float32)
    bn_beta = np.asarray(bn_beta, np.float32)
    bn_mean = np.asarray(bn_mean, np.float32)
    bn_var = np.asarray(bn_var, np.float32)
    B = x.shape[0]
    nc = build_nc()
    base = prep_core_inputs(x[0], offset_w, offset_b, conv_w, bn_gamma,
                            bn_beta, bn_mean, bn_var)
    in_maps = []
    for b in range(B):
        m = dict(base)
        if b > 0:
            xb = x[b]
            N = H * W
            m = dict(base)
            m["x_cm"] = xb.reshape(C, N).astype(np.float32)
            pm = np.ascontiguousarray(
                xb.reshape(C, N).T).astype(NPBF16)
            p = np.arange(N)
            m["x_pm4"] = np.concatenate(
                [pm[np.minimum(p + d, N - 1)] for d in (0, 1, W, W + 1)],
                axis=1)
        in_maps.append(m)
    res = bass_utils.run_bass_kernel_spmd(nc, in_maps,
                                          core_ids=list(range(B)))
    out = np.stack([res.results[b]["y"].reshape(C, H, W) for b in range(B)])
    return out.astype(np.float32)

